# revision 2
# baseline (speedup 1.0000x reference)
"""Multi-head attention (B=1, S=2048, H=1024, NH=16) on 8 trn2 NeuronCores.

Sharding: head-parallel. Core c owns heads {2c, 2c+1} (= 128 of the 1024
hidden dims). Each core computes its Q/K/V projection slices, the full
attention for its 2 heads, and a full-width partial of the output
projection (contraction over its 128 context dims). Host sums the 8
partials and adds the (host-folded) biases.

v2 schedule (engine-balanced, DMA-ordered):
  - q/k inputs arrive as 512-token panels; projections run panel-wise into
    a single rotating PSUM bank, with the per-dim bias folded into the DVE
    eviction (tensor_scalar add) - no bias matmuls.
  - attention h-major: per (h,j): S^T chunk (4x512 PE) -> mask-multiply
    (PSUM x fp8, DVE with every 3rd-ish half offloaded to GPSIMD/Pool) ->
    exp (Act, the critical engine: nothing else queues on Act until the
    last exp) -> PV (65-wide, ones-column denominator), software-pipelined
    by one j.  j0/j1 split their exp into 1024-halves so Act starts as
    soon as q panels 0-1 have landed.
  - V projection chunks + h0's epilogue (normalize/transpose/evict) are
    interleaved into j-loop slots of h0/h1 respectively.
  - y projection is pipelined per token-panel right behind h1's epilogue,
    with evictions round-robined over Act/DVE/Pool and eager output DMA.

Precision: matmuls bf16 with fp32 PSUM accumulation; 0/1 mask in fp8
(exact). Softmax runs without max-subtraction: exponent is (q.k/8)*M ~
N(0, 0.33^2), so exp never overflows.
"""

import math

import numpy as np
import ml_dtypes

BF16 = ml_dtypes.bfloat16
FP8 = ml_dtypes.float8_e4m3
S, H, NH, DK = 2048, 1024, 16, 64
NCORES = 8
HPC = NH // NCORES          # heads per core = 2
DPC = HPC * DK              # head dims per core = 128
KC = H // 128               # contraction chunks = 8
TP = S // 512               # 512-wide token panels = 4
JC = S // 128               # 128-wide key chunks = 16
VA = DK + 1                 # v columns + ones column = 65

_CACHE = {}

# engine assignment for the 64 mask-multiply halves: Pool takes half1 of
# (h,j) when (h*JC+j) % 3 != 2  (~21 halves), DVE the rest.
def _pool_half1(h, j):
    return (h * JC + j) % 3 != 2


def _oslc(ic):
    """o_ps column offset for ic-th 65-wide slice: 7 slices per 512-fp32
    PSUM bank so no matmul crosses a bank boundary."""
    b, r = divmod(ic, 7)
    return b * 512 + r * VA


def _build_program():
    """Build + compile the (identical) per-core Bass program."""
    from contextlib import ExitStack

    import concourse.bacc as bacc
    import concourse.bass as bass_mod
    import concourse.tile as tile
    from concourse import mybir

    dt = mybir.dt
    AF = mybir.ActivationFunctionType
    ALU = mybir.AluOpType
    f8 = dt.float8e4

    nc = bacc.Bacc("TRN2", target_bir_lowering=False, debug=False)

    qT_d = nc.dram_tensor("qT", [H, S], dt.bfloat16, kind="ExternalInput").ap()
    kT_d = nc.dram_tensor("kT", [H, S], dt.bfloat16, kind="ExternalInput").ap()
    vT_d = nc.dram_tensor("vT", [H, S], dt.bfloat16, kind="ExternalInput").ap()
    maskT_d = nc.dram_tensor("maskT", [S, S], f8, kind="ExternalInput").ap()
    wq_d = nc.dram_tensor("wq", [128, KC * DPC], dt.bfloat16, kind="ExternalInput").ap()
    wk_d = nc.dram_tensor("wk", [128, KC * DPC], dt.bfloat16, kind="ExternalInput").ap()
    wv_d = nc.dram_tensor("wv", [128, KC * DPC], dt.bfloat16, kind="ExternalInput").ap()
    wo_d = nc.dram_tensor("wo", [DPC, H], dt.bfloat16, kind="ExternalInput").ap()
    bq_d = nc.dram_tensor("bq", [DPC, 1], dt.float32, kind="ExternalInput").ap()
    bk_d = nc.dram_tensor("bk", [DPC, 1], dt.float32, kind="ExternalInput").ap()
    id_d = nc.dram_tensor("ident", [128, 128], dt.bfloat16, kind="ExternalInput").ap()
    yT_d = nc.dram_tensor("yT", [H, S], dt.bfloat16, kind="ExternalOutput").ap()

    with tile.TileContext(nc) as tc, ExitStack() as ctx:
        cp = ctx.enter_context(tc.tile_pool(name="const", bufs=1))
        xin_p = ctx.enter_context(tc.tile_pool(name="xin", bufs=3))
        vin_p = ctx.enter_context(tc.tile_pool(name="vin", bufs=3))
        sm_p = ctx.enter_context(tc.tile_pool(name="sm", bufs=3))
        e_p = ctx.enter_context(tc.tile_pool(name="ex", bufs=3))
        ot_p = ctx.enter_context(tc.tile_pool(name="otok", bufs=2))
        rc_p = ctx.enter_context(tc.tile_pool(name="recip", bufs=3))
        # PSUM: aux 1 bank (proj panels / pv chunks / transposes),
        # s 2x2 banks, o 3 banks -> 8 total
        aux_p = ctx.enter_context(tc.tile_pool(name="ps_aux", bufs=1, space="PSUM"))
        s_p = ctx.enter_context(tc.tile_pool(name="ps_s", bufs=2, space="PSUM"))
        o_p = ctx.enter_context(tc.tile_pool(name="ps_o", bufs=1, space="PSUM"))

        # ---- DMA order: ident/wq/wk, k_p0, q_p0, m0, q_p1, m1, q_p2, m2,
        # q_p3, m3, wv/wo/bias, v0, v1, k_p1, m4, v2, v3, k_p2, m5, m6,
        # v4, v5, k_p3, m7, m8, v6..., then m/v stream ----
        ident = cp.tile([128, 128], dt.bfloat16, tag="ident")
        nc.sync.dma_start(out=ident, in_=id_d)
        w_sb = {}
        for name, d in (("wq", wq_d), ("wk", wk_d)):
            w = cp.tile([128, KC * DPC], dt.bfloat16, tag=name, name=name)
            nc.sync.dma_start(out=w, in_=d)
            w_sb[name] = w

        qT_sb = cp.tile([128, S], dt.bfloat16, tag="qTs")
        kT_sb = cp.tile([128, S], dt.bfloat16, tag="kTs")
        vaug = cp.tile([128, JC * (HPC * VA)], dt.bfloat16, tag="vaug")
        m_sb = [cp.tile([128, S], f8, tag=f"mj{j}", name=f"mj{j}") for j in range(JC)]
        oT_sb = [cp.tile([128, 512], dt.bfloat16, tag=f"oTp{p}", name=f"oTp{p}")
                 for p in range(TP)]
        y_sb = [cp.tile([128, S], dt.bfloat16, tag=f"ysb{nn}", name=f"ysb{nn}")
                for nn in range(KC)]

        xin = {}          # (pre, p) -> panel tile
        vin = [None] * JC

        def dma_panel(pre, x_d, p):
            xt = xin_p.tile([128, KC * 512], dt.bfloat16, tag="xin",
                            name=f"x{pre}{p}")
            nc.sync.dma_start(
                out=xt.rearrange("p (c i) -> p c i", c=KC),
                in_=x_d[:, p * 512 : (p + 1) * 512].rearrange(
                    "(c p) i -> p c i", p=128
                ),
            )
            xin[pre, p] = xt

        def dma_mask(j):
            nc.sync.dma_start(out=m_sb[j], in_=maskT_d[j * 128 : (j + 1) * 128, :])

        def dma_v(t):
            vt = vin_p.tile([128, KC * 128], dt.bfloat16, tag="vin", name=f"v{t}")
            nc.sync.dma_start(
                out=vt.rearrange("p (c i) -> p c i", c=KC),
                in_=vT_d[:, t * 128 : (t + 1) * 128].rearrange(
                    "(c p) i -> p c i", p=128
                ),
            )
            vin[t] = vt

        dma_panel("k", kT_d, 0)
        dma_panel("q", qT_d, 0)
        dma_mask(0)
        dma_panel("q", qT_d, 1)
        dma_mask(1)
        dma_panel("q", qT_d, 2)
        dma_mask(2)
        dma_panel("q", qT_d, 3)
        dma_mask(3)
        for name, d in (("wv", wv_d),):
            w = cp.tile([128, KC * DPC], dt.bfloat16, tag=name, name=name)
            nc.sync.dma_start(out=w, in_=d)
            w_sb[name] = w
        wo_sb = cp.tile([128, H], dt.bfloat16, tag="wo")
        nc.sync.dma_start(out=wo_sb, in_=wo_d)
        bq_sb = cp.tile([DPC, 1], dt.float32, tag="bq")
        nc.sync.dma_start(out=bq_sb, in_=bq_d)
        bk_sb = cp.tile([DPC, 1], dt.float32, tag="bk")
        nc.sync.dma_start(out=bk_sb, in_=bk_d)
        dma_v(0)
        dma_v(1)
        dma_panel("k", kT_d, 1)
        dma_mask(4)
        dma_v(2)
        dma_v(3)
        dma_panel("k", kT_d, 2)
        dma_mask(5)
        dma_mask(6)
        dma_v(4)
        dma_v(5)
        dma_panel("k", kT_d, 3)
        dma_mask(7)
        dma_mask(8)
        dma_v(6)
        dma_v(7)
        for j in range(9, JC):
            dma_mask(j)
            dma_v(j - 1)
        dma_v(15)

        # ones columns of vaug, once (strided memset on Pool)
        ones_cols = bass_mod.AP(
            tensor=vaug.tensor,
            offset=vaug.offset + DK,
            ap=[vaug.ap[0], [VA, JC * HPC], [1, 1]],
        )
        nc.gpsimd.memset(ones_cols, 1.0)

        # ---- panel-wise projections (PE: aux bank; evict: DVE + bias) ----
        def proj_panel(pre, wname, b_sb, dest, p):
            ps = aux_p.tile([128, 512], dt.float32, tag="aux", name=f"pp{pre}{p}")
            for kk in range(KC):
                nc.tensor.matmul(
                    ps,
                    lhsT=w_sb[wname][:, kk * DPC : (kk + 1) * DPC],
                    rhs=xin[pre, p][:, kk * 512 : (kk + 1) * 512],
                    start=(kk == 0),
                    stop=(kk == KC - 1),
                )
            nc.vector.tensor_scalar(
                dest[:, p * 512 : (p + 1) * 512], ps, b_sb, None, ALU.add
            )

        def v_proj_chunk(t):
            ps = aux_p.tile([128, DPC], dt.float32, tag="aux", name=f"pv{t}")
            for kk in range(KC):
                nc.tensor.matmul(
                    ps,
                    lhsT=vin[t][:, kk * 128 : (kk + 1) * 128],
                    rhs=w_sb["wv"][:, kk * DPC : (kk + 1) * DPC],
                    start=(kk == 0),
                    stop=(kk == KC - 1),
                )
            base = t * (HPC * VA)
            for hh in range(HPC):
                nc.gpsimd.tensor_copy(
                    vaug[:, base + hh * VA : base + hh * VA + DK],
                    ps[:, hh * DK : (hh + 1) * DK],
                )

        proj_panel("k", "wk", bk_sb, kT_sb, 0)
        proj_panel("q", "wq", bq_sb, qT_sb, 0)
        proj_panel("q", "wq", bq_sb, qT_sb, 1)
        proj_panel("q", "wq", bq_sb, qT_sb, 2)
        proj_panel("q", "wq", bq_sb, qT_sb, 3)
        proj_panel("k", "wk", bk_sb, kT_sb, 1)
        v_proj_chunk(0)
        v_proj_chunk(1)
        proj_panel("k", "wk", bk_sb, kT_sb, 2)
        v_proj_chunk(2)
        proj_panel("k", "wk", bk_sb, kT_sb, 3)
        v_proj_chunk(3)

        # ---- attention ----
        import itertools

        def s_half(h, j, half):
            """S^T chunk j, token half -> PSUM tile [128, 1024]."""
            hs = h * DK
            ps = s_p.tile([128, 1024], dt.float32, tag="sps",
                          name=f"s{h}_{j}_{half}")
            for q in range(2):
                pi = half * 2 + q
                nc.tensor.matmul(
                    ps[:, q * 512 : (q + 1) * 512],
                    lhsT=kT_sb[hs : hs + DK, j * 128 : (j + 1) * 128],
                    rhs=qT_sb[hs : hs + DK, pi * 512 : (pi + 1) * 512],
                    start=True,
                    stop=True,
                )
            return ps

        def mask_mul(h, j, half, ps, sm):
            args = (sm[:, half * 1024 : (half + 1) * 1024], ps,
                    m_sb[j][:, half * 1024 : (half + 1) * 1024], ALU.mult)
            if half == 1 and _pool_half1(h, j):
                nc.gpsimd.tensor_tensor(*args)
            else:
                nc.vector.tensor_tensor(*args)

        def pv_mms(h, j, et, o_ps):
            for ic in range(JC):
                nc.tensor.matmul(
                    o_ps[:, _oslc(ic) : _oslc(ic) + VA],
                    lhsT=et[:, ic * 128 : (ic + 1) * 128],
                    rhs=vaug[:, j * (HPC * VA) + h * VA : j * (HPC * VA) + (h + 1) * VA],
                    start=(j == 0 and ic % 7 == 0),
                    stop=(j == JC - 1 and (ic % 7 == 6 or ic == JC - 1)),
                )

        # h0 epilogue pieces, interleaved into h1's j-loop. Each entry is a
        # closure; pop a few per j slot.
        epi_q = []

        def norm_bank(h, o_ps, b, ot_big):
            n_ic = (7, 7, 2)[b]
            rc = rc_p.tile([128, 8], dt.float32, tag="rc", name=f"rc{h}_{b}")
            den = bass_mod.AP(
                tensor=o_ps.tensor,
                offset=o_ps.offset + b * 512 + DK,
                ap=[o_ps.ap[0], [VA, n_ic]],
            )
            nc.vector.reciprocal(rc[:, :n_ic], den)
            src_ap = bass_mod.AP(
                tensor=o_ps.tensor,
                offset=o_ps.offset + b * 512,
                ap=[o_ps.ap[0], [VA, n_ic], [1, DK]],
            )
            rcb = bass_mod.AP(
                tensor=rc.tensor,
                offset=rc.offset,
                ap=[rc.ap[0], [1, n_ic], [0, DK]],
            )
            dst = ot_big[:, b * 7 * DK : (b * 7 + n_ic) * DK].rearrange(
                "p (a d) -> p a d", d=DK
            )
            nc.vector.tensor_mul(dst, src_ap, rcb)

        def tp_ot(h, ic, ot_big, psum_pool, psum_tag):
            hs = h * DK
            ot = ot_big[:, ic * DK : (ic + 1) * DK]
            tp = psum_pool.tile([DK, 128], dt.bfloat16, tag=psum_tag,
                                name=f"tp{h}_{ic}")
            nc.tensor.transpose(tp, ot, ident)
            dst = oT_sb[ic // 4][hs : hs + DK, (ic % 4) * 128 : (ic % 4 + 1) * 128]
            if ic % 2 == 0:
                nc.gpsimd.tensor_copy(dst, tp)
            else:
                nc.vector.tensor_copy(dst, tp)

        o_ps_h = [None, None]
        for h in range(HPC):
            o_ps = o_p.tile([128, 1536], dt.float32, tag="ops", name=f"ops{h}")
            o_ps_h[h] = o_ps
            pend = None  # (j, et) whose PV matmuls are not yet emitted
            for j in range(JC):
                sm = sm_p.tile([128, S], dt.bfloat16, tag="sm", name=f"sm{h}_{j}")
                et = e_p.tile([128, S], dt.bfloat16, tag="et", name=f"et{h}_{j}")
                split_exp = h == 0 and j < 2
                for half in range(2):
                    ps = s_half(h, j, half)
                    mask_mul(h, j, half, ps, sm)
                    if split_exp:
                        nc.scalar.activation(
                            et[:, half * 1024 : (half + 1) * 1024],
                            sm[:, half * 1024 : (half + 1) * 1024],
                            AF.Exp, scale=1.0 / math.sqrt(DK),
                        )
                if not split_exp:
                    nc.scalar.activation(et, sm, AF.Exp, scale=1.0 / math.sqrt(DK))
                # software pipeline: PE emits S(j+1) before PV(j); the V
                # projection (h0) / h0 epilogue (h1) rides the same slot.
                if pend is not None:
                    if h == 0:
                        if pend[0] + 4 < JC:
                            v_proj_chunk(pend[0] + 4)
                    else:
                        for _ in range(3):
                            if epi_q:
                                epi_q.pop(0)()
                    pv_mms(h, pend[0], pend[1], o_ps)
                pend = (j, et)
            if h == 0:
                pv_mms(h, pend[0], pend[1], o_ps)
                # queue h0's epilogue for interleave into h1's loop
                ot_big0 = ot_p.tile([128, JC * DK], dt.bfloat16, tag="ot",
                                    name="otb0")
                for b in range(3):
                    epi_q.append(lambda b=b: norm_bank(0, o_ps_h[0], b, ot_big0))
                for ic in range(JC):
                    epi_q.append(lambda ic=ic: tp_ot(0, ic, ot_big0, aux_p, "aux"))
            else:
                pv_mms(h, pend[0], pend[1], o_ps)
                while epi_q:
                    epi_q.pop(0)()

        # ---- h1 epilogue + y projection, pipelined per token panel ----
        ot_big1 = ot_p.tile([128, JC * DK], dt.bfloat16, tag="ot", name="otb1")
        ecyc = itertools.cycle(("act", "dve", "pool"))
        norm_done = set()
        for p in range(TP):
            for ic in range(4 * p, 4 * p + 4):
                b = ic // 7
                if b not in norm_done:
                    norm_done.add(b)
                    norm_bank(1, o_ps_h[1], b, ot_big1)
                # alternate transpose psum between aux and the free s banks
                if ic % 2 == 0:
                    tp_ot(1, ic, ot_big1, s_p, "sps")
                else:
                    tp_ot(1, ic, ot_big1, aux_p, "aux")
            for nn in range(KC):
                y_ps = s_p.tile([128, 1024], dt.float32, tag="sps",
                                name=f"y{p}_{nn}")
                nc.tensor.matmul(
                    y_ps[:, 0:512],
                    lhsT=wo_sb[:, nn * 128 : (nn + 1) * 128],
                    rhs=oT_sb[p],
                    start=True,
                    stop=True,
                )
                dst = y_sb[nn][:, p * 512 : (p + 1) * 512]
                eng = next(ecyc)
                if eng == "act":
                    nc.scalar.activation(dst, y_ps[:, 0:512], AF.Copy)
                elif eng == "dve":
                    nc.vector.tensor_copy(dst, y_ps[:, 0:512])
                else:
                    nc.gpsimd.tensor_copy(dst, y_ps[:, 0:512])
                if p == 1:
                    nc.sync.dma_start(
                        out=yT_d[nn * 128 : (nn + 1) * 128, 0:1024],
                        in_=y_sb[nn][:, 0:1024],
                    )
                elif p == TP - 1:
                    nc.sync.dma_start(
                        out=yT_d[nn * 128 : (nn + 1) * 128, 1024:2048],
                        in_=y_sb[nn][:, 1024:2048],
                    )

    nc.compile()
    return nc


def get_program():
    if "nc" not in _CACHE:
        _CACHE["nc"] = _build_program()
    return _CACHE["nc"]


def _wshuf(wT):
    """[1024 k, 128 n] -> [128 p, KC*128] with chunk kk at cols kk*128."""
    return np.ascontiguousarray(
        wT.reshape(KC, 128, DPC).transpose(1, 0, 2).reshape(128, KC * DPC)
    ).astype(BF16)


def make_in_maps(query, key, value, attention_mask, Wq, bq, Wk, bk, Wv, Wo):
    """Host-side sharding: per-core input dicts."""
    qT = np.ascontiguousarray(np.asarray(query, np.float32)[0].T).astype(BF16)
    kT = np.ascontiguousarray(np.asarray(key, np.float32)[0].T).astype(BF16)
    vT = np.ascontiguousarray(np.asarray(value, np.float32)[0].T).astype(BF16)
    maskT = np.ascontiguousarray(
        np.asarray(attention_mask, np.float32)[0, 0].T
    ).astype(FP8)

    in_maps = []
    for c in range(NCORES):
        ns = slice(c * DPC, (c + 1) * DPC)
        in_maps.append(
            {
                "qT": qT,
                "kT": kT,
                "vT": vT,
                "maskT": maskT,
                "wq": _wshuf(np.asarray(Wq, np.float32)[ns].T),
                "wk": _wshuf(np.asarray(Wk, np.float32)[ns].T),
                "wv": _wshuf(np.asarray(Wv, np.float32)[ns].T),
                "wo": np.ascontiguousarray(np.asarray(Wo, np.float32)[:, ns].T).astype(BF16),
                "bq": np.ascontiguousarray(np.asarray(bq, np.float32)[ns, None]),
                "bk": np.ascontiguousarray(np.asarray(bk, np.float32)[ns, None]),
                "ident": np.eye(128, dtype=BF16),
            }
        )
    return in_maps


def combine_outputs(results, Wv_bias, Wo, bo):
    """Sum per-core partial yT's (bf16 -> fp32), add host-folded biases."""
    acc = np.zeros((H, S), np.float32)
    for r in results:
        acc += r["yT"].astype(np.float32)
    bias = np.asarray(bo, np.float32) + np.asarray(Wv_bias, np.float32) @ np.asarray(
        Wo, np.float32
    ).T
    return (acc.T + bias[None, :]).astype(np.float32)[None]


def kernel(
    query,
    key,
    value,
    attention_mask,
    Wq,
    bq,
    Wk,
    bk,
    Wv,
    bv,
    Wo,
    bo,
    head,
    hidden_size,
):
    from concourse.bass_utils import run_bass_kernel_spmd

    nc = get_program()
    in_maps = make_in_maps(
        query, key, value, attention_mask, Wq, bq, Wk, bk, Wv, Wo
    )
    res = run_bass_kernel_spmd(nc, in_maps, list(range(NCORES)))
    return combine_outputs(res.results, bv, Wo, bo)


# revision 3
# speedup vs baseline: 1.0523x; 1.0523x over previous
"""Multi-head attention (B=1, S=2048, H=1024, NH=16) on 8 trn2 NeuronCores.

Sharding: head-parallel. Core c owns heads {2c, 2c+1} (= 128 of the 1024
hidden dims). Each core computes its Q/K/V projection slices, the full
attention for its 2 heads, and a full-width partial of the output
projection (contraction over its 128 context dims). Host sums the 8
partials and adds the (host-folded) biases.

v2 schedule (engine-balanced, DMA-ordered):
  - q/k inputs arrive as 512-token panels; projections run panel-wise into
    a single rotating PSUM bank, with the per-dim bias folded into the DVE
    eviction (tensor_scalar add) - no bias matmuls.
  - attention h-major: per (h,j): S^T chunk (4x512 PE) -> mask-multiply
    (PSUM x fp8) split DVE[0:1024], Pool[1024:1536], DVE[1536:2048] ->
    exp (Act, the critical engine: nothing else queues on Act until the
    last exp) -> PV (65-wide, ones-column denominator), software-pipelined
    by one j.  j0/j1 interleave their token-half exps so Act starts as
    soon as q panels 0-1 have landed.
  - V projection chunks + h0's epilogue (normalize/transpose/evict) are
    interleaved into j-loop slots of h0/h1 respectively.
  - y projection is pipelined per token-panel right behind h1's epilogue,
    with evictions round-robined over Act/DVE/Pool and eager output DMA.

Precision: matmuls bf16 with fp32 PSUM accumulation; 0/1 mask in fp8
(exact). Softmax runs without max-subtraction: exponent is (q.k/8)*M ~
N(0, 0.33^2), so exp never overflows.
"""

import math

import numpy as np
import ml_dtypes

BF16 = ml_dtypes.bfloat16
FP8 = ml_dtypes.float8_e4m3
S, H, NH, DK = 2048, 1024, 16, 64
NCORES = 8
HPC = NH // NCORES          # heads per core = 2
DPC = HPC * DK              # head dims per core = 128
KC = H // 128               # contraction chunks = 8
TP = S // 512               # 512-wide token panels = 4
JC = S // 128               # 128-wide key chunks = 16
VA = DK + 1                 # v columns + ones column = 65

_CACHE = {}


def _oslc(ic):
    """o_ps column offset for ic-th 65-wide slice: 7 slices per 512-fp32
    PSUM bank so no matmul crosses a bank boundary."""
    b, r = divmod(ic, 7)
    return b * 512 + r * VA


def _build_program():
    """Build + compile the (identical) per-core Bass program."""
    from contextlib import ExitStack

    import concourse.bacc as bacc
    import concourse.bass as bass_mod
    import concourse.tile as tile
    from concourse import mybir

    dt = mybir.dt
    AF = mybir.ActivationFunctionType
    ALU = mybir.AluOpType
    f8 = dt.float8e4

    nc = bacc.Bacc("TRN2", target_bir_lowering=False, debug=False)

    qT_d = nc.dram_tensor("qT", [H, S], dt.bfloat16, kind="ExternalInput").ap()
    kT_d = nc.dram_tensor("kT", [H, S], dt.bfloat16, kind="ExternalInput").ap()
    vT_d = nc.dram_tensor("vT", [H, S], dt.bfloat16, kind="ExternalInput").ap()
    maskT_d = nc.dram_tensor("maskT", [S, S], f8, kind="ExternalInput").ap()
    wq_d = nc.dram_tensor("wq", [128, KC * DPC], dt.bfloat16, kind="ExternalInput").ap()
    wk_d = nc.dram_tensor("wk", [128, KC * DPC], dt.bfloat16, kind="ExternalInput").ap()
    wv_d = nc.dram_tensor("wv", [128, KC * DPC], dt.bfloat16, kind="ExternalInput").ap()
    wo_d = nc.dram_tensor("wo", [DPC, H], dt.bfloat16, kind="ExternalInput").ap()
    bq_d = nc.dram_tensor("bq", [DPC, 1], dt.float32, kind="ExternalInput").ap()
    bk_d = nc.dram_tensor("bk", [DPC, 1], dt.float32, kind="ExternalInput").ap()
    id_d = nc.dram_tensor("ident", [128, 128], dt.bfloat16, kind="ExternalInput").ap()
    yT_d = nc.dram_tensor("yT", [H, S], dt.bfloat16, kind="ExternalOutput").ap()

    with tile.TileContext(nc) as tc, ExitStack() as ctx:
        cp = ctx.enter_context(tc.tile_pool(name="const", bufs=1))
        xin_p = ctx.enter_context(tc.tile_pool(name="xin", bufs=3))
        vin_p = ctx.enter_context(tc.tile_pool(name="vin", bufs=3))
        sm_p = ctx.enter_context(tc.tile_pool(name="sm", bufs=3))
        e_p = ctx.enter_context(tc.tile_pool(name="ex", bufs=4))
        ot_p = ctx.enter_context(tc.tile_pool(name="otok", bufs=2))
        rc_p = ctx.enter_context(tc.tile_pool(name="recip", bufs=3))
        # PSUM: aux 1 bank (proj panels / pv chunks / transposes),
        # s 2x2 banks, o 3 banks -> 8 total
        aux_p = ctx.enter_context(tc.tile_pool(name="ps_aux", bufs=1, space="PSUM"))
        s_p = ctx.enter_context(tc.tile_pool(name="ps_s", bufs=2, space="PSUM"))
        o_p = ctx.enter_context(tc.tile_pool(name="ps_o", bufs=1, space="PSUM"))

        ident = cp.tile([128, 128], dt.bfloat16, tag="ident")
        nc.sync.dma_start(out=ident, in_=id_d)
        w_sb = {}
        for name, d in (("wq", wq_d), ("wk", wk_d)):
            w = cp.tile([128, KC * DPC], dt.bfloat16, tag=name, name=name)
            nc.sync.dma_start(out=w, in_=d)
            w_sb[name] = w

        qT_sb = cp.tile([128, S], dt.bfloat16, tag="qTs")
        kT_sb = cp.tile([128, S], dt.bfloat16, tag="kTs")
        vaug = cp.tile([128, JC * (HPC * VA)], dt.bfloat16, tag="vaug")
        m_sb = [cp.tile([128, S], f8, tag=f"mj{j}", name=f"mj{j}") for j in range(JC)]
        oT_sb = [cp.tile([128, 512], dt.bfloat16, tag=f"oTp{p}", name=f"oTp{p}")
                 for p in range(TP)]
        y_sb = [cp.tile([128, S], dt.bfloat16, tag=f"ysb{nn}", name=f"ysb{nn}")
                for nn in range(KC)]

        xin = {}          # (pre, p) -> panel tile
        vin8 = [None] * KC  # 256-token v tiles

        def dma_panel(pre, x_d, p):
            xt = xin_p.tile([128, KC * 512], dt.bfloat16, tag="xin",
                            name=f"x{pre}{p}")
            nc.sync.dma_start(
                out=xt.rearrange("p (c i) -> p c i", c=KC),
                in_=x_d[:, p * 512 : (p + 1) * 512].rearrange(
                    "(c p) i -> p c i", p=128
                ),
            )
            xin[pre, p] = xt

        def dma_mask(j):
            nc.sync.dma_start(out=m_sb[j], in_=maskT_d[j * 128 : (j + 1) * 128, :])

        def dma_v(g):
            """256-token v tile g (covers proj chunks 2g, 2g+1)."""
            vt = vin_p.tile([128, KC * 256], dt.bfloat16, tag="vin", name=f"v{g}")
            nc.sync.dma_start(
                out=vt.rearrange("p (c i) -> p c i", c=KC),
                in_=vT_d[:, g * 256 : (g + 1) * 256].rearrange(
                    "(c p) i -> p c i", p=128
                ),
            )
            vin8[g] = vt

        # DMA order tuned so q panels + early masks land first, then k
        # panels / v tiles / remaining masks stream just-in-time.
        dma_panel("k", kT_d, 0)
        dma_panel("q", qT_d, 0)
        dma_mask(0)
        dma_panel("q", qT_d, 1)
        for name, d in (("wv", wv_d),):
            w = cp.tile([128, KC * DPC], dt.bfloat16, tag=name, name=name)
            nc.sync.dma_start(out=w, in_=d)
            w_sb[name] = w
        wo_sb = cp.tile([128, H], dt.bfloat16, tag="wo")
        nc.sync.dma_start(out=wo_sb, in_=wo_d)
        bq_sb = cp.tile([DPC, 1], dt.float32, tag="bq")
        nc.sync.dma_start(out=bq_sb, in_=bq_d)
        bk_sb = cp.tile([DPC, 1], dt.float32, tag="bk")
        nc.sync.dma_start(out=bk_sb, in_=bk_d)
        dma_panel("q", qT_d, 2)
        dma_mask(1)
        dma_panel("q", qT_d, 3)
        dma_mask(2)
        dma_mask(3)
        dma_v(0)
        dma_v(1)
        dma_panel("k", kT_d, 1)
        dma_mask(4)
        dma_mask(5)
        dma_v(2)
        dma_panel("k", kT_d, 2)
        dma_mask(6)
        dma_mask(7)
        dma_v(3)
        dma_panel("k", kT_d, 3)
        dma_mask(8)
        dma_v(4)
        dma_mask(9)
        dma_mask(10)
        dma_v(5)
        dma_mask(11)
        dma_mask(12)
        dma_v(6)
        dma_mask(13)
        dma_mask(14)
        dma_v(7)
        dma_mask(15)

        # ones columns of vaug, once (strided memset on Pool)
        ones_cols = bass_mod.AP(
            tensor=vaug.tensor,
            offset=vaug.offset + DK,
            ap=[vaug.ap[0], [VA, JC * HPC], [1, 1]],
        )
        nc.gpsimd.memset(ones_cols, 1.0)

        # ---- panel-wise projections (PE: aux bank; evict: DVE + bias) ----
        def proj_panel(pre, wname, b_sb, dest, p):
            ps = aux_p.tile([128, 512], dt.float32, tag="aux", name=f"pp{pre}{p}")
            for kk in range(KC):
                nc.tensor.matmul(
                    ps,
                    lhsT=w_sb[wname][:, kk * DPC : (kk + 1) * DPC],
                    rhs=xin[pre, p][:, kk * 512 : (kk + 1) * 512],
                    start=(kk == 0),
                    stop=(kk == KC - 1),
                )
            nc.vector.tensor_scalar(
                dest[:, p * 512 : (p + 1) * 512], ps, b_sb, None, ALU.add
            )

        def v_proj_chunk(t):
            ps = aux_p.tile([128, DPC], dt.float32, tag="aux", name=f"pv{t}")
            g, half = divmod(t, 2)
            for kk in range(KC):
                nc.tensor.matmul(
                    ps,
                    lhsT=vin8[g][:, kk * 256 + half * 128 : kk * 256 + half * 128 + 128],
                    rhs=w_sb["wv"][:, kk * DPC : (kk + 1) * DPC],
                    start=(kk == 0),
                    stop=(kk == KC - 1),
                )
            base = t * (HPC * VA)
            for hh in range(HPC):
                nc.gpsimd.tensor_copy(
                    vaug[:, base + hh * VA : base + hh * VA + DK],
                    ps[:, hh * DK : (hh + 1) * DK],
                )

        proj_panel("k", "wk", bk_sb, kT_sb, 0)
        proj_panel("q", "wq", bq_sb, qT_sb, 0)
        proj_panel("q", "wq", bq_sb, qT_sb, 1)
        proj_panel("q", "wq", bq_sb, qT_sb, 2)
        proj_panel("q", "wq", bq_sb, qT_sb, 3)
        proj_panel("k", "wk", bk_sb, kT_sb, 1)
        v_proj_chunk(0)
        v_proj_chunk(1)
        proj_panel("k", "wk", bk_sb, kT_sb, 2)
        v_proj_chunk(2)
        v_proj_chunk(3)
        proj_panel("k", "wk", bk_sb, kT_sb, 3)

        # ---- attention ----
        import itertools

        def s_half(h, j, half):
            """S^T chunk j, token half -> PSUM tile [128, 1024]."""
            hs = h * DK
            ps = s_p.tile([128, 1024], dt.float32, tag="sps",
                          name=f"s{h}_{j}_{half}")
            for q in range(2):
                pi = half * 2 + q
                nc.tensor.matmul(
                    ps[:, q * 512 : (q + 1) * 512],
                    lhsT=kT_sb[hs : hs + DK, j * 128 : (j + 1) * 128],
                    rhs=qT_sb[hs : hs + DK, pi * 512 : (pi + 1) * 512],
                    start=True,
                    stop=True,
                )
            return ps

        def mask_mul(h, j, half, ps, sm):
            """half0 -> one DVE op; half1 -> Pool[0:512] + DVE[512:1024]."""
            mj = m_sb[j]
            if half == 0:
                nc.vector.tensor_tensor(
                    sm[:, 0:1024], ps, mj[:, 0:1024], ALU.mult
                )
            else:
                nc.gpsimd.tensor_tensor(
                    sm[:, 1024:1536], ps[:, 0:512], mj[:, 1024:1536], ALU.mult
                )
                nc.vector.tensor_tensor(
                    sm[:, 1536:2048], ps[:, 512:1024], mj[:, 1536:2048], ALU.mult
                )

        def pv_mms(h, j, et, o_ps):
            for ic in range(JC):
                nc.tensor.matmul(
                    o_ps[:, _oslc(ic) : _oslc(ic) + VA],
                    lhsT=et[:, ic * 128 : (ic + 1) * 128],
                    rhs=vaug[:, j * (HPC * VA) + h * VA : j * (HPC * VA) + (h + 1) * VA],
                    start=(j == 0 and ic % 7 == 0),
                    stop=(j == JC - 1 and (ic % 7 == 6 or ic == JC - 1)),
                )

        epi_q = []  # h0 epilogue closures, paced into h1's j slots

        def norm_bank(h, o_ps, b, ot_big):
            n_ic = (7, 7, 2)[b]
            rc = rc_p.tile([128, 8], dt.float32, tag="rc", name=f"rc{h}_{b}")
            den = bass_mod.AP(
                tensor=o_ps.tensor,
                offset=o_ps.offset + b * 512 + DK,
                ap=[o_ps.ap[0], [VA, n_ic]],
            )
            nc.vector.reciprocal(rc[:, :n_ic], den)
            src_ap = bass_mod.AP(
                tensor=o_ps.tensor,
                offset=o_ps.offset + b * 512,
                ap=[o_ps.ap[0], [VA, n_ic], [1, DK]],
            )
            rcb = bass_mod.AP(
                tensor=rc.tensor,
                offset=rc.offset,
                ap=[rc.ap[0], [1, n_ic], [0, DK]],
            )
            dst = ot_big[:, b * 7 * DK : (b * 7 + n_ic) * DK].rearrange(
                "p (a d) -> p a d", d=DK
            )
            nc.vector.tensor_mul(dst, src_ap, rcb)

        def tp_ot(h, ic, ot_big, psum_pool, psum_tag):
            hs = h * DK
            ot = ot_big[:, ic * DK : (ic + 1) * DK]
            tp = psum_pool.tile([DK, 128], dt.bfloat16, tag=psum_tag,
                                name=f"tp{h}_{ic}")
            nc.tensor.transpose(tp, ot, ident)
            dst = oT_sb[ic // 4][hs : hs + DK, (ic % 4) * 128 : (ic % 4 + 1) * 128]
            nc.gpsimd.tensor_copy(dst, tp)

        def emit_sme(h, j, sm, et, half):
            ps = s_half(h, j, half)
            mask_mul(h, j, half, ps, sm)
            if half == 0:
                nc.scalar.activation(
                    et[:, 0:1024], sm[:, 0:1024], AF.Exp, scale=1.0 / math.sqrt(DK)
                )
            else:
                nc.scalar.activation(
                    et[:, 1024:2048], sm[:, 1024:2048], AF.Exp,
                    scale=1.0 / math.sqrt(DK),
                )

        o_ps_h = [None, None]
        for h in range(HPC):
            o_ps = o_p.tile([128, 1536], dt.float32, tag="ops", name=f"ops{h}")
            o_ps_h[h] = o_ps
            pend = []  # (j, et) whose PV matmuls are not yet emitted
            smet = {}
            if h == 0:
                # warm-up pair: interleave j0/j1 token-halves so Act's first
                # exps only need q panels 0-1 (+m0/m1)
                for j in (0, 1):
                    smet[j] = (
                        sm_p.tile([128, S], dt.bfloat16, tag="sm", name=f"sm0_{j}"),
                        e_p.tile([128, S], dt.bfloat16, tag="et", name=f"et0_{j}"),
                    )
                for half in range(2):
                    for j in (0, 1):
                        emit_sme(h, j, smet[j][0], smet[j][1], half)
                for j in (0, 1):
                    pend.append((j, smet[j][1]))
                jstart = 2
            else:
                jstart = 0
            for j in range(jstart, JC):
                sm = sm_p.tile([128, S], dt.bfloat16, tag="sm", name=f"sm{h}_{j}")
                et = e_p.tile([128, S], dt.bfloat16, tag="et", name=f"et{h}_{j}")
                ps0 = s_half(h, j, 0)
                mask_mul(h, j, 0, ps0, sm)
                ps1 = s_half(h, j, 1)
                mask_mul(h, j, 1, ps1, sm)
                nc.scalar.activation(et, sm, AF.Exp, scale=1.0 / math.sqrt(DK))
                # pend slot: v-proj (h0) / h0-epilogue pieces (h1), then PV
                if pend:
                    pj, pet = pend.pop(0)
                    if h == 0:
                        if pj + 4 < JC:
                            v_proj_chunk(pj + 4)
                    else:
                        npop = 2 if len(epi_q) > 2 * (JC - 1 - j) else 1
                        for _ in range(npop):
                            if epi_q:
                                epi_q.pop(0)()
                    pv_mms(h, pj, pet, o_ps)
                pend.append((j, et))
            for pj, pet in pend:
                pv_mms(h, pj, pet, o_ps)
            if h == 0:
                # queue h0's epilogue for interleave into h1's loop:
                # norms first (h1's PV blocks on them via o-pool WAR)
                ot_big0 = ot_p.tile([128, JC * DK], dt.bfloat16, tag="ot",
                                    name="otb0")
                for b in range(3):
                    epi_q.append(lambda b=b: norm_bank(0, o_ps_h[0], b, ot_big0))
                for ic in range(JC):
                    epi_q.append(lambda ic=ic: tp_ot(0, ic, ot_big0, aux_p, "aux"))
            else:
                while epi_q:
                    epi_q.pop(0)()

        # ---- h1 epilogue + y projection, pipelined per token panel ----
        ot_big1 = ot_p.tile([128, JC * DK], dt.bfloat16, tag="ot", name="otb1")
        ecyc = itertools.cycle(("act", "dve", "pool"))
        norm_done = set()
        for p in range(TP):
            for ic in range(4 * p, 4 * p + 4):
                b = ic // 7
                if b not in norm_done:
                    norm_done.add(b)
                    norm_bank(1, o_ps_h[1], b, ot_big1)
                # alternate transpose psum between aux and the free s banks
                if ic % 2 == 0:
                    tp_ot(1, ic, ot_big1, s_p, "sps")
                else:
                    tp_ot(1, ic, ot_big1, aux_p, "aux")
            for nn in range(KC):
                y_ps = s_p.tile([128, 1024], dt.float32, tag="sps",
                                name=f"y{p}_{nn}")
                nc.tensor.matmul(
                    y_ps[:, 0:512],
                    lhsT=wo_sb[:, nn * 128 : (nn + 1) * 128],
                    rhs=oT_sb[p],
                    start=True,
                    stop=True,
                )
                dst = y_sb[nn][:, p * 512 : (p + 1) * 512]
                eng = next(ecyc)
                if eng == "act":
                    nc.scalar.activation(dst, y_ps[:, 0:512], AF.Copy)
                elif eng == "dve":
                    nc.vector.tensor_copy(dst, y_ps[:, 0:512])
                else:
                    nc.gpsimd.tensor_copy(dst, y_ps[:, 0:512])
                if p == 1:
                    nc.sync.dma_start(
                        out=yT_d[nn * 128 : (nn + 1) * 128, 0:1024],
                        in_=y_sb[nn][:, 0:1024],
                    )
                elif p == TP - 1:
                    nc.sync.dma_start(
                        out=yT_d[nn * 128 : (nn + 1) * 128, 1024:2048],
                        in_=y_sb[nn][:, 1024:2048],
                    )

    nc.compile()
    return nc


def get_program():
    if "nc" not in _CACHE:
        _CACHE["nc"] = _build_program()
    return _CACHE["nc"]


def _wshuf(wT):
    """[1024 k, 128 n] -> [128 p, KC*128] with chunk kk at cols kk*128."""
    return np.ascontiguousarray(
        wT.reshape(KC, 128, DPC).transpose(1, 0, 2).reshape(128, KC * DPC)
    ).astype(BF16)


def make_in_maps(query, key, value, attention_mask, Wq, bq, Wk, bk, Wv, Wo):
    """Host-side sharding: per-core input dicts."""
    qT = np.ascontiguousarray(np.asarray(query, np.float32)[0].T).astype(BF16)
    kT = np.ascontiguousarray(np.asarray(key, np.float32)[0].T).astype(BF16)
    vT = np.ascontiguousarray(np.asarray(value, np.float32)[0].T).astype(BF16)
    maskT = np.ascontiguousarray(
        np.asarray(attention_mask, np.float32)[0, 0].T
    ).astype(FP8)

    in_maps = []
    for c in range(NCORES):
        ns = slice(c * DPC, (c + 1) * DPC)
        in_maps.append(
            {
                "qT": qT,
                "kT": kT,
                "vT": vT,
                "maskT": maskT,
                "wq": _wshuf(np.asarray(Wq, np.float32)[ns].T),
                "wk": _wshuf(np.asarray(Wk, np.float32)[ns].T),
                "wv": _wshuf(np.asarray(Wv, np.float32)[ns].T),
                "wo": np.ascontiguousarray(np.asarray(Wo, np.float32)[:, ns].T).astype(BF16),
                "bq": np.ascontiguousarray(np.asarray(bq, np.float32)[ns, None]),
                "bk": np.ascontiguousarray(np.asarray(bk, np.float32)[ns, None]),
                "ident": np.eye(128, dtype=BF16),
            }
        )
    return in_maps


def combine_outputs(results, Wv_bias, Wo, bo):
    """Sum per-core partial yT's (bf16 -> fp32), add host-folded biases."""
    acc = np.zeros((H, S), np.float32)
    for r in results:
        acc += r["yT"].astype(np.float32)
    bias = np.asarray(bo, np.float32) + np.asarray(Wv_bias, np.float32) @ np.asarray(
        Wo, np.float32
    ).T
    return (acc.T + bias[None, :]).astype(np.float32)[None]


def kernel(
    query,
    key,
    value,
    attention_mask,
    Wq,
    bq,
    Wk,
    bk,
    Wv,
    bv,
    Wo,
    bo,
    head,
    hidden_size,
):
    from concourse.bass_utils import run_bass_kernel_spmd

    nc = get_program()
    in_maps = make_in_maps(
        query, key, value, attention_mask, Wq, bq, Wk, bk, Wv, Wo
    )
    res = run_bass_kernel_spmd(nc, in_maps, list(range(NCORES)))
    return combine_outputs(res.results, bv, Wo, bo)
